# revision 1
# baseline (speedup 1.0000x reference)
"""MultiHeadSelection Trainium2 kernel.

scores[b,i,j,p] = sum_k tanh(x[b,i]@u_a[:,k] + x[b,j]@w_a[:,k] + b_s[k]) * v[k,p]

Shapes (hardcoded): x [8,256,768], u_a/w_a [768,256], b_s [256], v [256,50]
-> out [8,256,256,50] float32.

Sharding: data-parallel over batch, one batch element per NeuronCore (8 cores).
Each core:
  stage 1 (tiny): left_T[k,i] = (x_b @ u_a)^T, right_T[k,j] = (x_b @ w_a)^T
                  via PE matmuls with k on psum partitions (lhsT = weights
                  [h,k] chunk, rhs = x^T [h,*]); bias_all = left_T + b_s.
  stage 2 (hot):  for each i: pre[k,j] = right_T[k,j] + bias_all[k,i]
                  (DVE tensor_scalar, bf16 4x mode), tanh on ACT in big
                  FD=IB*256 ops, then PE matmuls lhsT=tanh[k, j-chunk]
                  (128-col bf16 weights -> FWL), rhs = v[k-chunk] bf16,
                  accumulated over the 2 k-chunks into psum [j_local, p].
                  psum -> SBUF staging (DVE) -> one 819KB DMA per 16-i block.
"""

import numpy as np
from contextlib import ExitStack

import concourse.bass as bass
import concourse.mybir as mybir
import concourse.tile as tile
from concourse import bacc

B, S, H, K, P = 8, 256, 768, 256, 50
NCORES = 8
IB = 16            # i-block size (ACT op free dim = IB*S = 4096)
GRP = 4            # i's per psum output tile ([128, GRP*2*50] = 1600B/bank)
KC = K // 128      # 2 k-chunks
HC = H // 128      # 6 h-chunks
JC = S // 128      # 2 j-chunks

F32 = mybir.dt.float32
BF16 = mybir.dt.bfloat16


def _build_nc(reps=1, ablate=()):
    ablate = set(ablate)
    # reps>1 repeats the whole computation on-device (same inputs/outputs) —
    # used only for timing: wall(R=3) - wall(R=1) isolates device time from
    # the per-call host/axon dispatch overhead.
    # Bacc (not raw Bass): its compile() pass splits multi-semaphore waits
    # into EventSemaphore instructions — TRN2 engine instructions hold 1 wait.
    nc = bacc.Bacc("TRN2", target_bir_lowering=False, debug=False,
                   enable_partition_id=False)

    xb = nc.dram_tensor("xb", [S, H], F32, kind="ExternalInput").ap()
    ua = nc.dram_tensor("ua", [H, K], F32, kind="ExternalInput").ap()
    wa = nc.dram_tensor("wa", [H, K], F32, kind="ExternalInput").ap()
    bs = nc.dram_tensor("bs", [K], F32, kind="ExternalInput").ap()
    vv = nc.dram_tensor("vv", [K, P], F32, kind="ExternalInput").ap()
    sc = nc.dram_tensor("scores", [S, S, P], F32, kind="ExternalOutput").ap()

    with ExitStack() as ctx:
        tc = ctx.enter_context(tile.TileContext(nc))
        singles = ctx.enter_context(tc.tile_pool(name="singles", bufs=1))
        work = ctx.enter_context(tc.tile_pool(name="work", bufs=2))
        outp = ctx.enter_context(tc.tile_pool(name="outp", bufs=2))

        # ---- constants ----
        v_bf = singles.tile([128, KC, P], BF16)
        for kc in range(KC):
            nc.gpsimd.dma_start(out=v_bf[:, kc, :], in_=vv[kc * 128:(kc + 1) * 128, :])
        bs_dma = singles.tile([128, KC], F32)
        for kc in range(KC):
            nc.sync.dma_start(out=bs_dma[:, kc:kc + 1], in_=bs[kc * 128:(kc + 1) * 128])
        # Bounce through a DVE copy so the DMA-completion wait lands on the
        # copy, not on the single-wait-slot TensorScalarPtr that consumes it.
        bs_col = singles.tile([128, KC], F32)
        nc.vector.tensor_copy(out=bs_col, in_=bs_dma)

        r_bf = singles.tile([128, KC, S], BF16)       # right_T, bf16
        bias_all = singles.tile([128, KC, S], F32)    # left_T + b_s, fp32

        # ---- stage 1 ----
        with tc.tile_pool(name="s1", bufs=1) as s1, \
             tc.tile_pool(name="s1d", bufs=1, space="DRAM") as s1d, \
             tc.tile_pool(name="ps1", bufs=2, space="PSUM") as ps1:
            u_bf = s1.tile([128, HC, K], BF16)
            w_bf = s1.tile([128, HC, K], BF16)
            for hc in range(HC):
                nc.gpsimd.dma_start(out=u_bf[:, hc, :], in_=ua[hc * 128:(hc + 1) * 128, :])
                nc.gpsimd.dma_start(out=w_bf[:, hc, :], in_=wa[hc * 128:(hc + 1) * 128, :])

            # x -> bf16 (DRAM scratch) -> transposed into SBUF as [h, i]
            xd = s1d.tile([S, H], BF16)
            nc.gpsimd.dma_start(out=xd, in_=xb)  # fp32 -> bf16 cast in DMA
            x_T = s1.tile([128, HC, S], BF16)
            for hc in range(HC):
                nc.sync.dma_start_transpose(out=x_T[:, hc, :], in_=xd[:, hc * 128:(hc + 1) * 128])

            for kc in range(KC):
                ps_r = ps1.tile([128, S], F32, tag="ps_r")
                ps_l = ps1.tile([128, S], F32, tag="ps_l")
                for hc in range(HC):
                    nc.tensor.matmul(ps_r, lhsT=w_bf[:, hc, kc * 128:(kc + 1) * 128],
                                     rhs=x_T[:, hc, :], start=(hc == 0), stop=(hc == HC - 1))
                for hc in range(HC):
                    nc.tensor.matmul(ps_l, lhsT=u_bf[:, hc, kc * 128:(kc + 1) * 128],
                                     rhs=x_T[:, hc, :], start=(hc == 0), stop=(hc == HC - 1))
                nc.vector.tensor_copy(out=r_bf[:, kc, :], in_=ps_r)
                # Two-step (copy then add) keeps the TensorScalarPtr at a
                # single semaphore wait: its ISA encoding has only one wait
                # slot, and a direct PSUM read would need PE + DMA waits.
                lt = s1.tile([128, S], F32, tag="lt")
                nc.vector.tensor_copy(out=lt, in_=ps_l)
                nc.vector.tensor_scalar_add(out=bias_all[:, kc, :], in0=lt,
                                            scalar1=bs_col[:, kc:kc + 1])

        # ---- stage 2 ----
        pso = ctx.enter_context(tc.tile_pool(name="pso", bufs=6, space="PSUM"))
        lin_scr = None
        if "lin_dma" in ablate or "relay" in ablate or "relay2" in ablate:
            lind = ctx.enter_context(tc.tile_pool(name="lind", bufs=1, space="DRAM"))
            lin_scr = lind.tile([S // IB, 128, IB, JC, P], F32)
        for blk in [b for _ in range(reps) for b in range(S // IB)]:
            pre = work.tile([128, KC, IB, S], BF16, tag="pre")
            th = work.tile([128, KC, IB, S], BF16, tag="th")
            # Absorb the buffer-reuse (WAR vs ACT) semaphore waits into this
            # memset: the TensorScalarPtr ISA struct has only one sync-wait
            # slot, so the preadds below must not carry cross-engine waits.
            nc.vector.memset(pre[:, 0, 0, 0:2], 0.0)
            for kc in range(KC):
                if "half_pre" in ablate:
                    for il in range(0, IB, 2):
                        i = blk * IB + il
                        # timing-only ablation: content is wrong, shape/rate match
                        nc.vector.tensor_scalar_add(out=pre[:, kc, il:il + 2, :],
                                                    in0=r_bf[:, 0:2, :],
                                                    scalar1=bias_all[:, kc, i:i + 1])
                else:
                    for il in range(IB):
                        i = blk * IB + il
                        nc.vector.tensor_scalar_add(out=pre[:, kc, il, :], in0=r_bf[:, kc, :],
                                                    scalar1=bias_all[:, kc, i:i + 1])
                if "no_act" not in ablate:
                    nc.scalar.activation(out=th[:, kc], in_=pre[:, kc],
                                         func=mybir.ActivationFunctionType.Tanh)
            src = pre if "no_act" in ablate else th
            kcs = [0] if "half_mm" in ablate else list(range(KC))
            if "v2" in ablate:
                # cols32 compute + 4-block output staging, stored in 8 large
                # DMAs per superblock: [32 part, 16 sb_g, 800B] each -> few
                # dma_starts (~1us fixed each) and 800B descriptor runs.
                nt, ng = 4, 4
                if blk % 4 == 0:
                    ost4 = outp.tile([128, 16, JC, nt, P], F32, tag="ost4")
                for g in range(IB // ng):
                    po = pso.tile([128, JC, nt, P], F32, tag="po")
                    for jc in range(JC):
                        for t in range(nt):
                            for kc in kcs:
                                for il_lo in range(ng):
                                    il = g * ng + il_lo
                                    wcols = src[:, kc, il, :].rearrange(
                                        "p (jc c t) -> p jc t c", jc=JC, t=nt)[:, jc, t, :]
                                    nc.tensor.matmul(
                                        po[32 * il_lo:32 * (il_lo + 1), jc, t, :],
                                        lhsT=wcols, rhs=v_bf[:, kc, :],
                                        start=(kc == kcs[0]), stop=(kc == kcs[-1]),
                                        tile_position=(0, 32 * il_lo))
                    nc.vector.tensor_copy(out=ost4[:, (blk % 4) * 4 + g], in_=po)
                if blk % 4 == 3:
                    i00 = (blk % (S // IB) - 3) * IB  # first i of the superblock
                    # out[c, sb_g, t, p] = scores[i00 + sb_g*4 + il_lo,
                    #                             jc*128 + 4c + t, p]
                    oap_all = sc[i00:i00 + 64].rearrange(
                        "(sbg il) (jc c t) p -> il c sbg jc (t p)", il=4, jc=JC, t=nt)
                    for il_lo in range(4):
                        for jc in range(JC):
                            nc.sync.dma_start(out=oap_all[il_lo, :, :, jc, :],
                                              in_=ost4[32 * il_lo:32 * (il_lo + 1), :, jc])
                continue
            nt = 4 if "cols32" in ablate else (2 if "cols64" in ablate else 0)
            if nt:
                # nt-way strided weight tiles (128//nt cols each), col-tiled
                # across psum groups: psum partition holds nt consecutive j ->
                # nt*200B-contiguous DRAM runs per descriptor.
                w = 128 // nt          # cols per tile
                ng = 128 // w          # psum col-groups per 128 partitions
                ost = outp.tile([128, IB // ng, JC, nt, P], F32, tag="ost")
                for g in range(IB // ng):
                    po = pso.tile([128, JC, nt, P], F32, tag="po")
                    for jc in range(JC):
                        for t in range(nt):
                            for kc in kcs:
                                for il_lo in range(ng):
                                    il = g * ng + il_lo
                                    wcols = src[:, kc, il, :].rearrange(
                                        "p (jc c t) -> p jc t c", jc=JC, t=nt)[:, jc, t, :]
                                    nc.tensor.matmul(
                                        po[w * il_lo:w * (il_lo + 1), jc, t, :],
                                        lhsT=wcols, rhs=v_bf[:, kc, :],
                                        start=(kc == kcs[0]), stop=(kc == kcs[-1]),
                                        tile_position=(0, w * il_lo))
                    nc.vector.tensor_copy(out=ost[:, g], in_=po)
            else:
                ost = outp.tile([128, IB, JC, P], F32, tag="ost")
                if "no_mm" in ablate:
                    nc.vector.memset(ost[:, 0, 0, 0:2], 0.0)
                else:
                    for g in range(IB // GRP):
                        po = pso.tile([128, GRP, JC, P], F32, tag="po")
                        for gi in range(GRP):
                            il = g * GRP + gi
                            for jc in range(JC):
                                for kc in kcs:
                                    nc.tensor.matmul(po[:, gi, jc, :],
                                                     lhsT=src[:, kc, il, jc * 128:(jc + 1) * 128],
                                                     rhs=v_bf[:, kc, :],
                                                     start=(kc == kcs[0]), stop=(kc == kcs[-1]))
                        nc.vector.tensor_copy(out=ost[:, g * GRP:(g + 1) * GRP], in_=po)
                if "gather" in ablate:
                    # SBUF->SBUF partition regroup (small descriptors are cheap
                    # off-HBM), so the HBM store runs with 6.4KB/partition runs.
                    lin = outp.tile([128, 32, P], F32, tag="lin")
                    for il in range(IB):
                        for jc in range(JC):
                            nc.sync.dma_start(
                                out=lin[8 * il + 4 * jc: 8 * il + 4 * jc + 4],
                                in_=ost[:, il, jc, :])
            if "no_dma" in ablate:
                pass
            elif "lin_dma" in ablate:
                nc.sync.dma_start(out=lin_scr[blk % (S // IB)], in_=ost)
            elif "relay" in ablate or "relay2" in ablate:
                # two-hop store: linear dump (line-rate writes), then a
                # DRAM->DRAM relayout whose writes are linear and whose reads
                # are 200B-strided (reads don't pay the sub-512B RMW penalty)
                b = blk % (S // IB)
                nc.sync.dma_start(out=lin_scr[b], in_=ost)
                eng = nc.scalar if "relay2" in ablate else nc.sync
                eng.dma_start(
                    out=sc[blk * IB:(blk + 1) * IB].rearrange("i (jc jl) p -> i jc jl p", jc=JC),
                    in_=lin_scr[b].rearrange("jl i jc p -> i jc jl p"),
                )
            elif "gather" in ablate:
                oap = sc[blk * IB:(blk + 1) * IB].rearrange(
                    "i (jc m jlo) p -> (i jc m) jlo p", jc=JC, m=4)
                nc.sync.dma_start(out=oap, in_=lin)
            elif nt:
                # ost[part=w*il_lo+c, g, jc, t, p] = scores[blk*IB+g*ng+il_lo,
                #                                           jc*128+nt*c+t, p]
                w = 128 // nt
                ng = 128 // w
                for g in range(IB // ng):
                    i0 = blk * IB + g * ng
                    oap = sc[i0:i0 + ng].rearrange(
                        "il_lo (jc c t) p -> il_lo c jc (t p)", jc=JC, t=nt)
                    for jc in range(JC):
                        nc.sync.dma_start(out=oap[:, :, jc, :], in_=ost[:, g, jc],
                                          single_packet="pkt" in ablate)
            else:
                oap = sc[blk * IB:(blk + 1) * IB].rearrange("i (jc jl) p -> jl i jc p", jc=JC)
                eng = nc.scalar if ("dma_split" in ablate and blk % 2) else nc.sync
                eng.dma_start(out=oap, in_=ost, single_packet="pkt" in ablate)

    return nc


_RUNNERS = {}


def _get_runner(reps=1, ablate=()):
    key = (reps, tuple(sorted(ablate)))
    if key in _RUNNERS:
        return _RUNNERS[key]
    import jax
    from jax.sharding import Mesh, PartitionSpec
    from jax.experimental.shard_map import shard_map
    from concourse.bass2jax import install_neuronx_cc_hook, _bass_exec_p

    install_neuronx_cc_hook()
    nc = _build_nc(reps=reps, ablate=ablate)
    if not nc.is_finalized():
        nc.finalize()

    in_names, out_names, out_avals = [], [], []
    for alloc in nc.m.functions[0].allocations:
        if not isinstance(alloc, mybir.MemoryLocationSet):
            continue
        if alloc.kind not in ("ExternalInput", "ExternalOutput"):
            continue
        name = alloc.memorylocations[0].name
        if alloc.kind == "ExternalInput":
            in_names.append(name)
        else:
            out_names.append(name)
            out_avals.append(jax.core.ShapedArray(tuple(alloc.tensor_shape),
                                                  mybir.dt.np(alloc.dtype)))
    n_params = len(in_names)
    all_in_names = tuple(in_names + out_names)

    def _body(*args):
        outs = _bass_exec_p.bind(
            *args,
            out_avals=tuple(out_avals),
            in_names=all_in_names,
            out_names=tuple(out_names),
            lowering_input_output_aliases=(),
            sim_require_finite=True,
            sim_require_nnan=True,
            nc=nc,
        )
        return tuple(outs)

    devices = jax.devices()[:NCORES]
    assert len(devices) == NCORES, f"need {NCORES} cores, got {len(devices)}"
    mesh = Mesh(np.asarray(devices), ("core",))
    nin = n_params + len(out_names)
    fn = jax.jit(
        shard_map(_body, mesh=mesh,
                  in_specs=(PartitionSpec("core"),) * nin,
                  out_specs=(PartitionSpec("core"),) * len(out_names),
                  check_rep=False),
        keep_unused=True,
    )
    _RUNNERS[key] = (fn, in_names, out_names, out_avals, mesh)
    return _RUNNERS[key]


def _concat_args(x, u_a, w_a, b_s, v, in_names, out_avals):
    x = np.ascontiguousarray(np.asarray(x, dtype=np.float32))
    u_a = np.asarray(u_a, dtype=np.float32)
    w_a = np.asarray(w_a, dtype=np.float32)
    b_s = np.asarray(b_s, dtype=np.float32)
    v = np.asarray(v, dtype=np.float32)
    per = {
        "xb": x.reshape(NCORES * S, H),
        "ua": np.tile(u_a, (NCORES, 1)),
        "wa": np.tile(w_a, (NCORES, 1)),
        "bs": np.tile(b_s, NCORES),
        "vv": np.tile(v, (NCORES, 1)),
    }
    args = [per[n] for n in in_names]
    args += [np.zeros((NCORES * a.shape[0], *a.shape[1:]), a.dtype) for a in out_avals]
    return args


def kernel(x, u_a, w_a, b_s, v):
    fn, in_names, out_names, out_avals, mesh = _get_runner()
    args = _concat_args(x, u_a, w_a, b_s, v, in_names, out_avals)
    outs = fn(*args)
    scores = np.asarray(outs[out_names.index("scores")])
    return scores.reshape(B, S, S, P)


def _timed_calls(reps, x, u_a, w_a, b_s, v, iters, ablate=()):
    import time
    import jax
    from jax.sharding import NamedSharding, PartitionSpec

    fn, in_names, out_names, out_avals, mesh = _get_runner(reps=reps, ablate=ablate)
    args = _concat_args(x, u_a, w_a, b_s, v, in_names, out_avals)
    sh = NamedSharding(mesh, PartitionSpec("core"))
    dargs = [jax.device_put(a, sh) for a in args]
    for _ in range(3):  # warmup (also triggers compile)
        outs = fn(*dargs)
    jax.block_until_ready(outs)
    times = []
    for _ in range(iters):
        t0 = time.perf_counter()
        out = fn(*dargs)
        jax.block_until_ready(out)
        times.append(time.perf_counter() - t0)
    return times


def bench(x, u_a, w_a, b_s, v, iters=10, r_hi=5):
    """Estimate on-device time of one full computation.

    Runs NEFFs with the stage-2 loop executed once and r_hi times; the
    difference isolates device time from per-call host/axon dispatch
    overhead. Returns seconds for one computation (stage2 delta-based).
    """
    t1 = _timed_calls(1, x, u_a, w_a, b_s, v, iters)
    th = _timed_calls(r_hi, x, u_a, w_a, b_s, v, iters)
    t1m, thm = min(t1), min(th)
    stage2 = (thm - t1m) / (r_hi - 1)
    return stage2, dict(t_r1=t1m, t_rhi=thm, r_hi=r_hi,
                        med_r1=sorted(t1)[len(t1) // 2],
                        med_rhi=sorted(th)[len(th) // 2])



# revision 3
# speedup vs baseline: 2.2462x; 2.2462x over previous
"""MultiHeadSelection Trainium2 kernel.

scores[b,i,j,p] = sum_k tanh(x[b,i]@u_a[:,k] + x[b,j]@w_a[:,k] + b_s[k]) * v[k,p]

Shapes (hardcoded): x [8,256,768], u_a/w_a [768,256], b_s [256], v [256,50]
-> out [8,256,256,50] float32.

Sharding: data-parallel over batch, one batch element per NeuronCore (8 cores).

Per-core dataflow (j-major so the output DMA is linear):
  stage 1: Lb[k,i] = (x_b @ u_a)^T + b_s   (bf16, k on partitions)
           R2[k,j,2] = (x_b @ w_a)^T replicated 2x along an inner dim
           (the x2 replication gives every tensor_tensor operand a packed
           2-byte inner dim -> DVE 2x mode)
  stage 2, for each block of JB=16 j's:
    pre[k,kc,j,i] = R2[k,j] + Lb[k,i]      one DVE tensor_tensor per kc,
                                           4096-elem ops in 2x mode
    th = tanh(pre)                         one 8192-elem ACT op per block
    for ic, oct:  psum[i(128), jl(8), p] += th[k, j, ic]^T @ v[kc]
                                           (i on psum partitions)
    gpsimd copies psum -> ost[i, j, p];    sync DMAs ost -> scores with
                                           3.2KB/partition contiguous runs
"""

import numpy as np
from contextlib import ExitStack

import concourse.bass as bass
import concourse.mybir as mybir
import concourse.tile as tile
from concourse import bacc

B, S, H, K, P = 8, 256, 768, 256, 50
NCORES = 8
JB = 16            # j's per stage-2 block
OCT = 8            # j's per psum bank ([128, 8*50] f32 = 1600B <= 2KB)
KC = K // 128      # 2 k-chunks
HC = H // 128      # 6 h-chunks
NB = S // JB       # 16 blocks

F32 = mybir.dt.float32
BF16 = mybir.dt.bfloat16


def _build_nc(reps=1, ablate=()):
    ablate = set(ablate)
    # reps>1 repeats the stage-2 loop on-device (same inputs/outputs) —
    # used only for timing: wall(R) - wall(1) isolates device time from
    # the per-call host/axon dispatch overhead.
    nc = bacc.Bacc("TRN2", target_bir_lowering=False, debug=False,
                   enable_partition_id=False)

    xb = nc.dram_tensor("xb", [S, H], F32, kind="ExternalInput").ap()
    ua = nc.dram_tensor("ua", [H, K], F32, kind="ExternalInput").ap()
    wa = nc.dram_tensor("wa", [H, K], F32, kind="ExternalInput").ap()
    bs = nc.dram_tensor("bs", [K], F32, kind="ExternalInput").ap()
    vv = nc.dram_tensor("vv", [K, P], F32, kind="ExternalInput").ap()
    sc = nc.dram_tensor("scores", [S, S, P], F32, kind="ExternalOutput").ap()

    with ExitStack() as ctx:
        tc = ctx.enter_context(tile.TileContext(nc))
        singles = ctx.enter_context(tc.tile_pool(name="singles", bufs=1))

        # ---- constants ----
        v_bf = singles.tile([128, KC, P], BF16)
        for kc in range(KC):
            nc.gpsimd.dma_start(out=v_bf[:, kc, :], in_=vv[kc * 128:(kc + 1) * 128, :])
        bs_dma = singles.tile([128, KC], F32)
        for kc in range(KC):
            nc.sync.dma_start(out=bs_dma[:, kc:kc + 1], in_=bs[kc * 128:(kc + 1) * 128])
        # Bounce through a DVE copy so the DMA-completion wait lands on the
        # copy, not on the single-wait-slot consumer.
        bs_col = singles.tile([128, KC], F32)
        nc.vector.tensor_copy(out=bs_col, in_=bs_dma)

        Lb = singles.tile([128, KC, S], BF16)      # left^T + b_s
        R2 = singles.tile([128, KC, S, 2], BF16)   # right^T, x2 replicated

        # ---- stage 1 ----
        with tc.tile_pool(name="s1", bufs=1) as s1, \
             tc.tile_pool(name="s1d", bufs=1, space="DRAM") as s1d, \
             tc.tile_pool(name="ps1", bufs=4, space="PSUM") as ps1:
            u_bf = s1.tile([128, HC, K], BF16)
            w_bf = s1.tile([128, HC, K], BF16)
            for hc in range(HC):
                nc.gpsimd.dma_start(out=u_bf[:, hc, :], in_=ua[hc * 128:(hc + 1) * 128, :])
                nc.gpsimd.dma_start(out=w_bf[:, hc, :], in_=wa[hc * 128:(hc + 1) * 128, :])

            # x -> bf16 (DRAM scratch) -> transposed into SBUF as [h, i]
            xd = s1d.tile([S, H], BF16)
            nc.gpsimd.dma_start(out=xd, in_=xb)  # fp32 -> bf16 cast in DMA
            x_T = s1.tile([128, HC, S], BF16)
            for hc in range(HC):
                nc.sync.dma_start_transpose(out=x_T[:, hc, :], in_=xd[:, hc * 128:(hc + 1) * 128])

            for kc in range(KC):
                ps_r = ps1.tile([128, S], F32, tag="ps_r")
                ps_l = ps1.tile([128, S], F32, tag="ps_l")
                for hc in range(HC):
                    nc.tensor.matmul(ps_r, lhsT=w_bf[:, hc, kc * 128:(kc + 1) * 128],
                                     rhs=x_T[:, hc, :], start=(hc == 0), stop=(hc == HC - 1))
                for hc in range(HC):
                    nc.tensor.matmul(ps_l, lhsT=u_bf[:, hc, kc * 128:(kc + 1) * 128],
                                     rhs=x_T[:, hc, :], start=(hc == 0), stop=(hc == HC - 1))
                # Lb = ps_l + b_s (ACT Identity with per-partition bias)
                nc.scalar.activation(out=Lb[:, kc, :], in_=ps_l,
                                     func=mybir.ActivationFunctionType.Identity,
                                     bias=bs_col[:, kc:kc + 1])
                # R2[k, j, il] = ps_r[k, j] for il in {0, 1}
                r_in = ps_r.unsqueeze(2).broadcast_to((128, S, 2))
                nc.scalar.activation(out=R2[:, kc], in_=r_in,
                                     func=mybir.ActivationFunctionType.Copy)

        # ---- stage 2 ----
        work = ctx.enter_context(tc.tile_pool(name="work", bufs=2))
        ostp = ctx.enter_context(tc.tile_pool(name="ostp", bufs=2))
        pso = ctx.enter_context(tc.tile_pool(name="pso", bufs=8, space="PSUM"))

        for blk in [b for _ in range(reps) for b in range(NB)]:
            j0 = blk * JB
            pre = work.tile([128, KC, JB, S], BF16, tag="pre")
            th = work.tile([128, KC, JB, S], BF16, tag="th")
            for kc in range(KC):
                # pre[k, j, ih, il] = R2[k, j, il] + Lb[k, ih*2 + il]
                # (ih, il) = i split 128x2 so every operand has a packed
                # 2-byte inner dim -> DVE 2x mode.
                in0 = R2[:, kc, j0:j0 + JB, :].unsqueeze(2).broadcast_to(
                    (128, JB, 128, 2))
                in1 = Lb[:, kc, :].rearrange("p (ih il) -> p ih il", il=2) \
                    .unsqueeze(1).broadcast_to((128, JB, 128, 2))
                outv = pre[:, kc].rearrange("p j (ih il) -> p j ih il", il=2)
                nc.vector.tensor_add(out=outv, in0=in0, in1=in1)
            if "no_act" not in ablate:
                nc.scalar.activation(out=th, in_=pre,
                                     func=mybir.ActivationFunctionType.Tanh)
            src = pre if "no_act" in ablate else th
            ost = ostp.tile([128, 2, JB, P], F32, tag="ost")
            for ic in range(2):
                for oc in range(JB // OCT):
                    po = pso.tile([128, OCT, P], F32, tag="po")
                    if "no_mm" not in ablate:
                        for jl in range(OCT):
                            jloc = oc * OCT + jl
                            for kc in range(KC):
                                nc.tensor.matmul(
                                    po[:, jl, :],
                                    lhsT=src[:, kc, jloc, ic * 128:(ic + 1) * 128],
                                    rhs=v_bf[:, kc, :],
                                    start=(kc == 0), stop=(kc == KC - 1))
                    # (GPSIMD cannot read PSUM on TRN2, so DVE evicts.)
                    nc.vector.tensor_copy(
                        out=ost[:, ic, oc * OCT:(oc + 1) * OCT, :], in_=po)
                if "no_dma" not in ablate:
                    nc.sync.dma_start(
                        out=sc[ic * 128:(ic + 1) * 128, j0:j0 + JB, :],
                        in_=ost[:, ic])

    return nc


_RUNNERS = {}


def _get_runner(reps=1, ablate=()):
    key = (reps, tuple(sorted(ablate)))
    if key in _RUNNERS:
        return _RUNNERS[key]
    import jax
    from jax.sharding import Mesh, PartitionSpec
    from jax.experimental.shard_map import shard_map
    from concourse.bass2jax import install_neuronx_cc_hook, _bass_exec_p

    install_neuronx_cc_hook()
    nc = _build_nc(reps=reps, ablate=ablate)
    if not nc.is_finalized():
        nc.finalize()

    in_names, out_names, out_avals = [], [], []
    for alloc in nc.m.functions[0].allocations:
        if not isinstance(alloc, mybir.MemoryLocationSet):
            continue
        if alloc.kind not in ("ExternalInput", "ExternalOutput"):
            continue
        name = alloc.memorylocations[0].name
        if alloc.kind == "ExternalInput":
            in_names.append(name)
        else:
            out_names.append(name)
            out_avals.append(jax.core.ShapedArray(tuple(alloc.tensor_shape),
                                                  mybir.dt.np(alloc.dtype)))
    n_params = len(in_names)
    all_in_names = tuple(in_names + out_names)

    def _body(*args):
        outs = _bass_exec_p.bind(
            *args,
            out_avals=tuple(out_avals),
            in_names=all_in_names,
            out_names=tuple(out_names),
            lowering_input_output_aliases=(),
            sim_require_finite=True,
            sim_require_nnan=True,
            nc=nc,
        )
        return tuple(outs)

    devices = jax.devices()[:NCORES]
    assert len(devices) == NCORES, f"need {NCORES} cores, got {len(devices)}"
    mesh = Mesh(np.asarray(devices), ("core",))
    nin = n_params + len(out_names)
    fn = jax.jit(
        shard_map(_body, mesh=mesh,
                  in_specs=(PartitionSpec("core"),) * nin,
                  out_specs=(PartitionSpec("core"),) * len(out_names),
                  check_rep=False),
        keep_unused=True,
    )
    _RUNNERS[key] = (fn, in_names, out_names, out_avals, mesh)
    return _RUNNERS[key]


def _concat_args(x, u_a, w_a, b_s, v, in_names, out_avals):
    x = np.ascontiguousarray(np.asarray(x, dtype=np.float32))
    u_a = np.asarray(u_a, dtype=np.float32)
    w_a = np.asarray(w_a, dtype=np.float32)
    b_s = np.asarray(b_s, dtype=np.float32)
    v = np.asarray(v, dtype=np.float32)
    per = {
        "xb": x.reshape(NCORES * S, H),
        "ua": np.tile(u_a, (NCORES, 1)),
        "wa": np.tile(w_a, (NCORES, 1)),
        "bs": np.tile(b_s, NCORES),
        "vv": np.tile(v, (NCORES, 1)),
    }
    args = [per[n] for n in in_names]
    args += [np.zeros((NCORES * a.shape[0], *a.shape[1:]), a.dtype) for a in out_avals]
    return args


def kernel(x, u_a, w_a, b_s, v):
    fn, in_names, out_names, out_avals, mesh = _get_runner()
    args = _concat_args(x, u_a, w_a, b_s, v, in_names, out_avals)
    outs = fn(*args)
    scores = np.asarray(outs[out_names.index("scores")])
    return scores.reshape(B, S, S, P)


def _timed_calls(reps, x, u_a, w_a, b_s, v, iters, ablate=()):
    import time
    import jax
    from jax.sharding import NamedSharding, PartitionSpec

    fn, in_names, out_names, out_avals, mesh = _get_runner(reps=reps, ablate=ablate)
    args = _concat_args(x, u_a, w_a, b_s, v, in_names, out_avals)
    sh = NamedSharding(mesh, PartitionSpec("core"))
    dargs = [jax.device_put(a, sh) for a in args]
    for _ in range(3):  # warmup (also triggers compile)
        outs = fn(*dargs)
    jax.block_until_ready(outs)
    times = []
    for _ in range(iters):
        t0 = time.perf_counter()
        out = fn(*dargs)
        jax.block_until_ready(out)
        times.append(time.perf_counter() - t0)
    return times


def bench(x, u_a, w_a, b_s, v, iters=10, r_hi=5):
    """Estimate on-device time of one full computation.

    Runs NEFFs with the stage-2 loop executed once and r_hi times; the
    difference isolates device time from per-call host/axon dispatch
    overhead. Returns seconds for one computation (stage2 delta-based).
    """
    t1 = _timed_calls(1, x, u_a, w_a, b_s, v, iters)
    th = _timed_calls(r_hi, x, u_a, w_a, b_s, v, iters)
    t1m, thm = min(t1), min(th)
    stage2 = (thm - t1m) / (r_hi - 1)
    return stage2, dict(t_r1=t1m, t_rhi=thm, r_hi=r_hi,
                        med_r1=sorted(t1)[len(t1) // 2],
                        med_rhi=sorted(th)[len(th) // 2])


# revision 7
# speedup vs baseline: 7.9048x; 3.5192x over previous
"""MultiHeadSelection Trainium2 kernel.

scores[b,i,j,p] = sum_k tanh(x[b,i]@u_a[:,k] + x[b,j]@w_a[:,k] + b_s[k]) * v[k,p]

Shapes (hardcoded): x [8,256,768], u_a/w_a [768,256], b_s [256], v [256,50]
-> out [8,256,256,50] float32.

Sharding: data-parallel over batch, one batch element per NeuronCore (8 cores).

Per-core dataflow (j-major so the output DMA is linear):
  stage 1: Lb[k,i] = (x_b @ u_a)^T + b_s   (bf16, k on partitions)
           R2[k,j,2] = (x_b @ w_a)^T replicated 2x along an inner dim
           (the x2 replication gives every tensor_tensor operand a packed
           2-byte inner dim -> DVE 2x mode)
  stage 2, for each block of JB=16 j's:
    pre[k,kc,j,i] = R2[k,j] + Lb[k,i]      one DVE tensor_tensor per kc,
                                           4096-elem ops in 2x mode
    th = tanh(pre)                         one 8192-elem ACT op per block
    for ic, oct:  psum[i(128), jl(8), p] += th[k, j, ic]^T @ v[kc]
                                           (i on psum partitions)
    gpsimd copies psum -> ost[i, j, p];    sync DMAs ost -> scores with
                                           3.2KB/partition contiguous runs
"""

import numpy as np
from contextlib import ExitStack

import concourse.bass as bass
import concourse.mybir as mybir
import concourse.tile as tile
from concourse import bacc

B, S, H, K, P = 8, 256, 768, 256, 50
NCORES = 8
JB = 16            # j's per stage-2 block
OCT = 8            # j's per psum bank ([128, 8*50] f32 = 1600B <= 2KB)
KC = K // 128      # 2 k-chunks
HC = H // 128      # 6 h-chunks
NB = S // JB       # 16 blocks

F32 = mybir.dt.float32
BF16 = mybir.dt.bfloat16


def _build_nc(reps=1, ablate=()):
    ablate = set(ablate)
    # reps>1 repeats the stage-2 loop on-device (same inputs/outputs) —
    # used only for timing: wall(R) - wall(1) isolates device time from
    # the per-call host/axon dispatch overhead.
    nc = bacc.Bacc("TRN2", target_bir_lowering=False, debug=False,
                   enable_partition_id=False)

    xb = nc.dram_tensor("xb", [S, H], F32, kind="ExternalInput").ap()
    ua = nc.dram_tensor("ua", [H, K], F32, kind="ExternalInput").ap()
    wa = nc.dram_tensor("wa", [H, K], F32, kind="ExternalInput").ap()
    bs = nc.dram_tensor("bs", [K], F32, kind="ExternalInput").ap()
    vv = nc.dram_tensor("vv", [K, P], F32, kind="ExternalInput").ap()
    sc = nc.dram_tensor("scores", [S, S, P], F32, kind="ExternalOutput").ap()

    with ExitStack() as ctx:
        tc = ctx.enter_context(tile.TileContext(nc))
        singles = ctx.enter_context(tc.tile_pool(name="singles", bufs=1))

        # ---- constants ----
        v_bf = singles.tile([128, KC, P], BF16)
        for kc in range(KC):
            nc.gpsimd.dma_start(out=v_bf[:, kc, :], in_=vv[kc * 128:(kc + 1) * 128, :])
        bs_dma = singles.tile([128, KC], F32)
        for kc in range(KC):
            nc.sync.dma_start(out=bs_dma[:, kc:kc + 1], in_=bs[kc * 128:(kc + 1) * 128])
        # Bounce through a DVE copy so the DMA-completion wait lands on the
        # copy, not on the single-wait-slot consumer.
        bs_col = singles.tile([128, KC], F32)
        nc.vector.tensor_copy(out=bs_col, in_=bs_dma)

        Lb = singles.tile([128, KC, S], BF16)      # left^T + b_s
        R2 = singles.tile([128, KC, S, 2], BF16)   # right^T, x2 replicated

        # ---- stage 1 ----
        with tc.tile_pool(name="s1", bufs=1) as s1, \
             tc.tile_pool(name="s1d", bufs=1, space="DRAM") as s1d, \
             tc.tile_pool(name="ps1", bufs=4, space="PSUM") as ps1:
            # x -> bf16 (DRAM scratch) first: it gates the transposes, which
            # gate the stage-1 matmuls. Two chunks so transposes start early.
            xd = s1d.tile([S, H], BF16)
            nc.gpsimd.dma_start(out=xd[:, :H // 2], in_=xb[:, :H // 2])
            nc.gpsimd.dma_start(out=xd[:, H // 2:], in_=xb[:, H // 2:])
            # u/w on the idle vector/scalar DGE queues, in parallel with xd.
            u_bf = s1.tile([128, HC, K], BF16)
            w_bf = s1.tile([128, HC, K], BF16)
            # One strided cast-DMA per weight tensor (cast requires gpsimd's
            # SWDGE; batching amortizes the per-instruction DGE overhead).
            nc.gpsimd.dma_start(out=u_bf, in_=ua.rearrange("(hc p) k -> p hc k", p=128))
            nc.gpsimd.dma_start(out=w_bf, in_=wa.rearrange("(hc p) k -> p hc k", p=128))

            x_T = s1.tile([128, HC, S], BF16)
            for hc in range(HC):
                eng = nc.sync if hc % 2 == 0 else nc.scalar
                eng.dma_start_transpose(out=x_T[:, hc, :], in_=xd[:, hc * 128:(hc + 1) * 128])

            for kc in range(KC):
                ps_r = ps1.tile([128, S], F32, tag="ps_r")
                ps_l = ps1.tile([128, S], F32, tag="ps_l")
                for hc in range(HC):
                    nc.tensor.matmul(ps_r, lhsT=w_bf[:, hc, kc * 128:(kc + 1) * 128],
                                     rhs=x_T[:, hc, :], start=(hc == 0), stop=(hc == HC - 1))
                for hc in range(HC):
                    nc.tensor.matmul(ps_l, lhsT=u_bf[:, hc, kc * 128:(kc + 1) * 128],
                                     rhs=x_T[:, hc, :], start=(hc == 0), stop=(hc == HC - 1))
                # Lb = ps_l + b_s (ACT Identity with per-partition bias)
                nc.scalar.activation(out=Lb[:, kc, :], in_=ps_l,
                                     func=mybir.ActivationFunctionType.Identity,
                                     bias=bs_col[:, kc:kc + 1])
                # R2[k, j, il] = ps_r[k, j] for il in {0, 1}
                r_in = ps_r.unsqueeze(2).broadcast_to((128, S, 2))
                nc.scalar.activation(out=R2[:, kc], in_=r_in,
                                     func=mybir.ActivationFunctionType.Copy)

        # ---- stage 2 ----
        work = ctx.enter_context(tc.tile_pool(name="work", bufs=2))
        ostp = ctx.enter_context(tc.tile_pool(name="ostp", bufs=2))
        pso = ctx.enter_context(tc.tile_pool(name="pso", bufs=8, space="PSUM"))

        for blk in [b for _ in range(reps) for b in range(NB)]:
            j0 = blk * JB
            pre = work.tile([128, KC, JB, S], BF16, tag="pre")
            th = work.tile([128, KC, JB, S], BF16, tag="th")
            for kc in range(KC):
                # pre[k, j, ih, il] = R2[k, j, il] + Lb[k, ih*2 + il]
                # (ih, il) = i split 128x2 so every operand has a packed
                # 2-byte inner dim -> DVE 2x mode. The last quarter of kc=1
                # goes to the otherwise-idle GPSIMD so the DVE finishes a
                # block comfortably inside ACT's tanh time (no ACT stalls).
                def _pieces(kc):
                    if kc == 0:
                        return [(nc.vector, 0, JB)]
                    return [(nc.vector, 0, JB - 4), (nc.gpsimd, JB - 4, JB)]
                for eng, ja, jb_ in _pieces(kc):
                    nj = jb_ - ja
                    in0 = R2[:, kc, j0 + ja:j0 + jb_, :].unsqueeze(2) \
                        .broadcast_to((128, nj, 128, 2))
                    in1 = Lb[:, kc, :].rearrange("p (ih il) -> p ih il", il=2) \
                        .unsqueeze(1).broadcast_to((128, nj, 128, 2))
                    outv = pre[:, kc, ja:jb_].rearrange(
                        "p j (ih il) -> p j ih il", il=2)
                    eng.tensor_add(out=outv, in0=in0, in1=in1)
            if "no_act" not in ablate:
                nc.scalar.activation(out=th, in_=pre,
                                     func=mybir.ActivationFunctionType.Tanh)
            src = pre if "no_act" in ablate else th
            ost = ostp.tile([128, 2, JB, P], F32, tag="ost")
            for ic in range(2):
                for oc in range(JB // OCT):
                    po = pso.tile([128, OCT, P], F32, tag="po")
                    if "no_mm" not in ablate:
                        for jl in range(OCT):
                            jloc = oc * OCT + jl
                            for kc in range(KC):
                                nc.tensor.matmul(
                                    po[:, jl, :],
                                    lhsT=src[:, kc, jloc, ic * 128:(ic + 1) * 128],
                                    rhs=v_bf[:, kc, :],
                                    start=(kc == 0), stop=(kc == KC - 1))
                    # (GPSIMD cannot read PSUM on TRN2, so DVE evicts.)
                    nc.vector.tensor_copy(
                        out=ost[:, ic, oc * OCT:(oc + 1) * OCT, :], in_=po)
                if "no_dma" not in ablate:
                    nc.sync.dma_start(
                        out=sc[ic * 128:(ic + 1) * 128, j0:j0 + JB, :],
                        in_=ost[:, ic])

    return nc


_RUNNERS = {}


def _get_runner(reps=1, ablate=()):
    key = (reps, tuple(sorted(ablate)))
    if key in _RUNNERS:
        return _RUNNERS[key]
    import jax
    from jax.sharding import Mesh, PartitionSpec
    from jax.experimental.shard_map import shard_map
    from concourse.bass2jax import install_neuronx_cc_hook, _bass_exec_p

    install_neuronx_cc_hook()
    nc = _build_nc(reps=reps, ablate=ablate)
    if not nc.is_finalized():
        nc.finalize()

    in_names, out_names, out_avals = [], [], []
    for alloc in nc.m.functions[0].allocations:
        if not isinstance(alloc, mybir.MemoryLocationSet):
            continue
        if alloc.kind not in ("ExternalInput", "ExternalOutput"):
            continue
        name = alloc.memorylocations[0].name
        if alloc.kind == "ExternalInput":
            in_names.append(name)
        else:
            out_names.append(name)
            out_avals.append(jax.core.ShapedArray(tuple(alloc.tensor_shape),
                                                  mybir.dt.np(alloc.dtype)))
    n_params = len(in_names)
    all_in_names = tuple(in_names + out_names)

    def _body(*args):
        outs = _bass_exec_p.bind(
            *args,
            out_avals=tuple(out_avals),
            in_names=all_in_names,
            out_names=tuple(out_names),
            lowering_input_output_aliases=(),
            sim_require_finite=True,
            sim_require_nnan=True,
            nc=nc,
        )
        return tuple(outs)

    devices = jax.devices()[:NCORES]
    assert len(devices) == NCORES, f"need {NCORES} cores, got {len(devices)}"
    mesh = Mesh(np.asarray(devices), ("core",))
    nin = n_params + len(out_names)
    fn = jax.jit(
        shard_map(_body, mesh=mesh,
                  in_specs=(PartitionSpec("core"),) * nin,
                  out_specs=(PartitionSpec("core"),) * len(out_names),
                  check_rep=False),
        keep_unused=True,
    )
    _RUNNERS[key] = (fn, in_names, out_names, out_avals, mesh)
    return _RUNNERS[key]


def _concat_args(x, u_a, w_a, b_s, v, in_names, out_avals):
    x = np.ascontiguousarray(np.asarray(x, dtype=np.float32))
    u_a = np.asarray(u_a, dtype=np.float32)
    w_a = np.asarray(w_a, dtype=np.float32)
    b_s = np.asarray(b_s, dtype=np.float32)
    v = np.asarray(v, dtype=np.float32)
    per = {
        "xb": x.reshape(NCORES * S, H),
        "ua": np.tile(u_a, (NCORES, 1)),
        "wa": np.tile(w_a, (NCORES, 1)),
        "bs": np.tile(b_s, NCORES),
        "vv": np.tile(v, (NCORES, 1)),
    }
    args = [per[n] for n in in_names]
    args += [np.zeros((NCORES * a.shape[0], *a.shape[1:]), a.dtype) for a in out_avals]
    return args


def kernel(x, u_a, w_a, b_s, v):
    fn, in_names, out_names, out_avals, mesh = _get_runner()
    args = _concat_args(x, u_a, w_a, b_s, v, in_names, out_avals)
    outs = fn(*args)
    scores = np.asarray(outs[out_names.index("scores")])
    return scores.reshape(B, S, S, P)


def _timed_calls(reps, x, u_a, w_a, b_s, v, iters, ablate=()):
    import time
    import jax
    from jax.sharding import NamedSharding, PartitionSpec

    fn, in_names, out_names, out_avals, mesh = _get_runner(reps=reps, ablate=ablate)
    args = _concat_args(x, u_a, w_a, b_s, v, in_names, out_avals)
    sh = NamedSharding(mesh, PartitionSpec("core"))
    dargs = [jax.device_put(a, sh) for a in args]
    for _ in range(3):  # warmup (also triggers compile)
        outs = fn(*dargs)
    jax.block_until_ready(outs)
    times = []
    for _ in range(iters):
        t0 = time.perf_counter()
        out = fn(*dargs)
        jax.block_until_ready(out)
        times.append(time.perf_counter() - t0)
    return times


def bench(x, u_a, w_a, b_s, v, iters=10, r_hi=5):
    """Estimate on-device time of one full computation.

    Runs NEFFs with the stage-2 loop executed once and r_hi times; the
    difference isolates device time from per-call host/axon dispatch
    overhead. Returns seconds for one computation (stage2 delta-based).
    """
    t1 = _timed_calls(1, x, u_a, w_a, b_s, v, iters)
    th = _timed_calls(r_hi, x, u_a, w_a, b_s, v, iters)
    t1m, thm = min(t1), min(th)
    stage2 = (thm - t1m) / (r_hi - 1)
    return stage2, dict(t_r1=t1m, t_rhi=thm, r_hi=r_hi,
                        med_r1=sorted(t1)[len(t1) // 2],
                        med_rhi=sorted(th)[len(th) // 2])
